# revision 82
# baseline (speedup 1.0000x reference)
"""AdaptiveConv2d Trainium2 kernel.

Reference computation (B=32, CIN=32, COUT=64, K=3, H=W=128, FIN=64):
    h   = relu(z @ w1.T + b1); h = relu(h @ w2.T + b2)
    aw  = relu(h @ w3.T + b3)                      # (B, 18496)
    kern = aw[:, :18432] -> (B, 64, 32, 3, 3)      # per-sample conv weights
    bias = aw[:, 18432:]                           # (B, 64)
    y = relu(conv2d_same(x, kern) + bias)          # (B, 64, 128, 128)

Strategy: pure data parallel over 8 NeuronCores, 4 samples per core.
Each core runs the kernel-generating MLP (all transposes/bias-folding are
pre-done on host via augmented weight matrices) and the per-sample convs.

The conv is expressed as 9 shifted matmuls (one per 3x3 tap) per 4-row
output tile, accumulated in PSUM. One matmul covers a SAMPLE PAIR:
lhsT is a (64, 128) block-diagonal weight tile (sample 2p in rows 0:32 /
cols 0:64, sample 2p+1 in rows 32:64 / cols 64:128), rhs is both samples'
zero-padded bf16 input (64 partitions x 4 rows x 128 cols at the tap
offset), and the two pairs run concurrently in the PE array as 64-row
tiles (tile_position=(64*pair, 0)). That streams the minimum possible
N-columns for this shape: 9 taps x 16K pixels x 2 pair-lanes at
K*M/(128*128) = 50% cell utilization = 2*288/128 = 4.5 column-slots
per output pixel pair, ~61 us of PE streaming per core.

Other key moves (mostly DMA-shaped, found via neuron-profile traces):
 - inputs are host-cast to bf16 and host-padded so every load is a big
   linear DMA spanning all 128 partitions (engine = partition group / 8,
   so <128-partition DMAs strand most of the 16 SDMA engines)
 - w3 is host-permuted so the generated weights come out in
   (cin-chunk, tap, cout) order, and host-reshaped to (128, 4672) so the
   final MLP layer runs as 4 concurrent 32-row PE tiles and its output
   lands across all partitions
 - the generated-weight rearrange into the block-diag layout is 16 small
   SBUF->SBUF DMAs spread over the sync/scalar/gpsimd queues
 - output is written in the SBUF staging layout (rowgroup, pair, 128,
   16, 128) -- pure linear per-partition 8KB descriptors reach ~26GB/s
   per DMA engine vs ~15GB/s for the strided (B, COUT, H, W) layout --
   and the host un-permutes afterwards
 - conv epilogue (bias+relu, PSUM->SBUF f32) alternates ScalarE and
   VectorE

Compute in bf16, accumulate/output f32. L2 rel err vs the f32 reference
is ~5e-3.
"""

import sys
import types

import numpy as np
import ml_dtypes

BF16 = ml_dtypes.bfloat16

B, CIN, COUT, KS, H, W, FIN = 32, 32, 64, 3, 128, 128, 64
L1, L2 = 20, 30
NKW = CIN * COUT * KS * KS  # 18432
NOUT = NKW + COUT  # 18496
N_CORES = 8
BS = B // N_CORES  # 4 samples per core
HP, WP = H + 2, W + 2  # 'same'-padded input


def _install_ntff_hook():
    """Make run_bass_kernel_spmd(trace=True) work under axon by providing
    the antenv.axon_hooks module the image lacks. Safe no-op on failure."""
    try:
        if "antenv.axon_hooks" in sys.modules:
            return
        import antenv

        mod = types.ModuleType("antenv.axon_hooks")
        mod._hook = None
        mod.set_axon_ntff_profile_hook = lambda h: setattr(mod, "_hook", h)
        mod.get_axon_ntff_profile_hook = lambda: mod._hook
        sys.modules["antenv.axon_hooks"] = mod
        antenv.axon_hooks = mod
        from trn_agent_boot.trn_boot import _ntff_profile_via_ctypes

        hook = _ntff_profile_via_ctypes("/opt/axon/libaxon_pjrt.so")
        if hook is not None:
            mod.set_axon_ntff_profile_hook(hook)
    except Exception:
        pass


def build_nc():
    import concourse.tile as tile
    from concourse import bacc, mybir

    dt = mybir.dt
    Relu = mybir.ActivationFunctionType.Relu

    nc = bacc.Bacc(
        "TRN2", target_bir_lowering=False, debug=False, num_devices=N_CORES
    )
    xa = nc.dram_tensor("xa", [BS * CIN, HP, WP], dt.bfloat16, kind="ExternalInput")
    zaT = nc.dram_tensor("zaT", [FIN + 1, BS], dt.bfloat16, kind="ExternalInput")
    w1a = nc.dram_tensor("w1a", [FIN + 1, L1], dt.bfloat16, kind="ExternalInput")
    w2a = nc.dram_tensor("w2a", [L1 + 1, L2], dt.bfloat16, kind="ExternalInput")
    # w3 split into 4 column-chunks stacked on partitions: row 32c+k is
    # (w3.T row k) of chunk c for k<30, row 32c+30 is b3 of chunk c,
    # row 32c+31 is zeros. Spans 128 partitions -> all 16 DMA engines.
    # Chunks align with groups of 8 input channels (8*576 = 4608 cols);
    # chunk 3 carries 64 extra cols for the conv bias; chunks 0-2 are
    # zero-padded to NCH so the matmul tiling is uniform.
    NCW = NKW // 4  # 4608 weight columns per chunk
    NCH = NCW + COUT  # 4672 including the bias tail (chunk 3 only)
    w3a = nc.dram_tensor("w3a", [128, NCH], dt.bfloat16, kind="ExternalInput")
    # device-friendly output layout: (rowgroup, pair, partition, RG, W);
    # partition p of pair holds sample 2*pair + p//64, cout p%64.
    # The host un-permutes to (BS, COUT, H, W).
    RG = 16
    out = nc.dram_tensor(
        "out", [H // RG, 2, 128, RG, W], dt.float32, kind="ExternalOutput"
    )

    with tile.TileContext(nc) as tc:
        with (
            tc.tile_pool(name="const", bufs=1) as cp,
            tc.tile_pool(name="outp", bufs=3) as op,
        ):
            xp = cp.tile([BS * CIN, HP, WP], dt.bfloat16)
            w3s = cp.tile([128, NCH], dt.bfloat16)
            # aw[32c+s, jj] = generated weight j' = c*4608 + jj (jj<4608);
            # rows with (p%32)>=BS are dead. Bias lives at [96+s, 4608:4672].
            aw = cp.tile([128, NCH], dt.bfloat16)
            # block-diagonal per-pair conv weights:
            # partitions [64p:64p+64] = pair p, free (tap, 2*COUT);
            # sample 2p at cols 0:64 (rows 0:32), sample 2p+1 at 64:128.
            wsb = cp.tile([BS * CIN, KS * KS, 2 * COUT], dt.bfloat16)
            zs = cp.tile([FIN + 1, BS], dt.bfloat16)
            w1s = cp.tile([FIN + 1, L1], dt.bfloat16)
            w2s = cp.tile([L1 + 1, L2], dt.bfloat16)
            h1a = cp.tile([L1 + 1, BS], dt.bfloat16)
            # 4 chunk-replicas of h2; 32 columns (only 0:BS meaningful) so
            # the final-layer matmuls write full 32-row psum blocks
            h2a = cp.tile([128, 32], dt.bfloat16)
            biasb = [
                cp.tile([128, 1], dt.bfloat16, name=f"biasb{p}") for p in range(2)
            ]
            biasf = [
                cp.tile([128, 1], dt.float32, name=f"biasf{p}") for p in range(2)
            ]

            # input DMAs: w3 first (it gates the MLP tail), then the small
            # weights, then x in full-partition row-chunks so all 16 DMA
            # engines engage.
            nc.sync.dma_start(w3s[:], w3a.ap())
            nc.sync.dma_start(zs[:], zaT.ap())
            nc.sync.dma_start(w1s[:], w1a.ap())
            nc.sync.dma_start(w2s[:], w2a.ap())
            XCH = 8
            for c in range(XCH):
                rl = (HP + XCH - 1) // XCH
                a, b = c * rl, min(HP, (c + 1) * rl)
                nc.sync.dma_start(xp[:, a:b, :], xa.ap()[:, a:b, :])
            # pre-warm the gpsimd SWDGE path so the weight-rearrange DMAs
            # on that queue don't pay its first-use drain
            nc.gpsimd.dma_start(biasb[0][0:1, 0:1], zaT.ap()[0:1, 0:1])

            # ---- MLP generating conv weights ----
            with tc.tile_pool(name="mlpp", bufs=1, space="PSUM") as mp:
                h1p = mp.tile([L1, BS], dt.float32)
                nc.tensor.matmul(h1p[:], w1s[:], zs[:], start=True, stop=True)
                nc.vector.memset(h1a[:], 1.0)
                nc.scalar.activation(h1a[0:L1, :], h1p[:], Relu)

                h2p = mp.tile([L2, BS], dt.float32)
                nc.tensor.matmul(h2p[:], w2s[:], h1a[:], start=True, stop=True)
                # h2 replicated into 4 chunk-blocks; rows 32c+30 stay 1.0
                # (bias feature), rows 32c+31 are 1.0 x zero w3 row = 0.
                # sample s lands in column 8*s so that the aw rows used by
                # the rearrange DMAs spread across all 16 DMA engine groups
                nc.vector.memset(h2a[:], 1.0)
                for c in range(4):
                    nc.scalar.activation(
                        h2a[32 * c : 32 * c + L2, 0 : 8 * BS : 8], h2p[:], Relu
                    )

                # final layer: 4 chunks run as concurrent PE row-tiles,
                # each writing 4 partitions of one full-width psum tile
                ntile = (NCH + 511) // 512
                for jt in range(ntile):
                    n0 = jt * 512
                    n1 = min(NCH, n0 + 512)
                    awp = mp.tile(
                        [128, n1 - n0], dt.float32, tag="awp", bufs=6, name="awp"
                    )
                    for c in range(4):
                        nc.tensor.matmul(
                            awp[32 * c : 32 * c + 32, :],
                            h2a[32 * c : 32 * c + 32, :],
                            w3s[32 * c : 32 * c + 32, n0:n1],
                            start=True,
                            stop=True,
                            tile_position=(32 * c, 32 * c),
                        )
                    # full-width relu; alternate engines for chain latency
                    if jt % 2 == 0:
                        nc.vector.tensor_scalar_max(aw[:, n0:n1], awp[:], 0.0)
                    else:
                        nc.scalar.activation(aw[:, n0:n1], awp[:], Relu)

            # ---- rearrange generated weights ----
            # aw[32c+s, ci8*576 + t*64 + co] (ci = 8c+ci8)
            #   -> wsb[32s+ci, t, 64*(s%2)+co]
            nc.vector.memset(wsb[:], 0.0)
            # 3-way split: ScalarE is idle in this window (conv epilogues
            # only start after the first psum group completes)
            dma_engs = [nc.sync, nc.scalar, nc.gpsimd]
            for s in range(BS):
                h64 = 64 * (s % 2)
                for c in range(4):
                    sp = 32 * c + 8 * s
                    src = aw[sp : sp + 1, 0:NCW].rearrange(
                        "p (c q) -> p c q", c=8, q=KS * KS * COUT
                    )
                    p0 = s * CIN + 8 * c
                    eng = dma_engs[(s * 4 + c) % 3]
                    eng.dma_start(wsb[p0 : p0 + 8, :, h64 : h64 + COUT], src)
            # per-sample bias -> per-pair (128,1) column vectors
            for pair in range(2):
                for hh in range(2):
                    s = pair * 2 + hh
                    beng = nc.gpsimd
                    beng.dma_start(
                        biasb[pair][64 * hh : 64 * hh + 64, 0:1],
                        aw[96 + 8 * s : 96 + 8 * s + 1, NCW:NCH],
                    )
                nc.vector.tensor_copy(biasf[pair][:], biasb[pair][:])

            # ---- conv: 8 rowgroups (16 rows) x 4 row-tiles x 9 taps x 2 pairs ----
            # One matmul covers a sample pair: lhsT (64, 128) block-diag,
            # rhs = both samples' x (64 partitions), out = full psum bank.
            with tc.tile_pool(name="cps", bufs=8, space="PSUM") as cps:
                for rg in range(H // RG):
                    obig = [
                        op.tile([128, RG, W], dt.float32, tag=f"ob{p}", name=f"ob{p}")
                        for p in range(2)
                    ]
                    for half in range(RG // 8):
                        pss = [
                            [
                                cps.tile([128, 4, W], dt.float32, tag="ps", name="ps")
                                for _ in range(2)
                            ]
                            for _ in range(2)
                        ]
                        for tap in range(KS * KS):
                            ky, kx = divmod(tap, KS)
                            for pt in range(2):
                                r0 = rg * RG + (half * 2 + pt) * 4
                                for pair in range(2):
                                    p0 = pair * 2 * CIN
                                    nc.tensor.matmul(
                                        pss[pt][pair][:],
                                        wsb[p0 : p0 + 2 * CIN, tap, :],
                                        xp[p0 : p0 + 2 * CIN, r0 + ky : r0 + ky + 4, kx : kx + W],
                                        start=(tap == 0),
                                        stop=(tap == KS * KS - 1),
                                        tile_position=(64 * pair, 0),
                                    )
                        for pt in range(2):
                            rr = (half * 2 + pt) * 4
                            for pair in range(2):
                                # bias+relu epilogue, alternating engines
                                if (half + pt) % 2 == 0:
                                    nc.scalar.activation(
                                        obig[pair][:, rr : rr + 4, :],
                                        pss[pt][pair][:],
                                        Relu,
                                        bias=biasf[pair][:],
                                    )
                                else:
                                    nc.vector.tensor_scalar(
                                        obig[pair][:, rr : rr + 4, :],
                                        pss[pt][pair][:],
                                        biasf[pair][:],
                                        0.0,
                                        mybir.AluOpType.add,
                                        mybir.AluOpType.max,
                                    )
                    if rg < H // RG - 1:
                        for pair in range(2):
                            eng = nc.sync if (rg * 2 + pair) % 2 == 0 else nc.gpsimd
                            eng.dma_start(out.ap()[rg, pair], obig[pair][:])
                    else:
                        # flush the final rowgroup in 8-row halves so the
                        # first half drains while the second half computes
                        for hrow in range(2):
                            for pair in range(2):
                                eng = nc.sync if pair == 0 else nc.gpsimd
                                eng.dma_start(
                                    out.ap()[rg, pair][:, 8 * hrow : 8 * hrow + 8, :],
                                    obig[pair][:, 8 * hrow : 8 * hrow + 8, :],
                                )

    nc.compile()
    return nc


def _host_prep(x, z, w1, b1, w2, b2, w3, b3):
    """Build per-core input maps (all np, bf16 where device expects bf16)."""
    x = np.asarray(x, np.float32)
    z = np.asarray(z, np.float32)
    w1 = np.asarray(w1, np.float32)
    b1 = np.asarray(b1, np.float32)
    w2 = np.asarray(w2, np.float32)
    b2 = np.asarray(b2, np.float32)
    w3 = np.asarray(w3, np.float32)
    b3 = np.asarray(b3, np.float32)

    w1a = np.concatenate([w1.T, b1[None, :]], axis=0).astype(BF16)  # (65, 20)
    w2a = np.concatenate([w2.T, b2[None, :]], axis=0).astype(BF16)  # (21, 30)

    # permute w3 rows: old j = co*288 + ci*9 + t  ->  j' = ci*576 + t*64 + co
    t = np.arange(KS * KS)
    ci = np.arange(CIN)
    co = np.arange(COUT)
    oldj = (
        co[None, None, :] * (CIN * KS * KS)
        + ci[:, None, None] * (KS * KS)
        + t[None, :, None]
    ).reshape(-1)
    w3flat = np.empty((L2 + 1, NOUT), np.float32)
    w3flat[0:L2, :NKW] = w3[oldj].T
    w3flat[L2, :NKW] = b3[oldj]
    w3flat[0:L2, NKW:] = w3[NKW:].T
    w3flat[L2, NKW:] = b3[NKW:]
    # split into 4 column-chunks stacked on partitions; chunk 3 also
    # carries the 64 bias columns at the tail (chunks 0-2 zero-padded)
    NCW = NKW // 4  # 4608
    NCH = NCW + COUT  # 4672
    w3a = np.zeros((128, NCH), np.float32)
    for c in range(4):
        w3a[32 * c : 32 * c + L2 + 1, 0:NCW] = w3flat[:, c * NCW : (c + 1) * NCW]
    w3a[96 : 96 + L2 + 1, NCW:NCH] = w3flat[:, NKW:NOUT]
    w3a = w3a.astype(BF16)

    in_maps = []
    for c in range(N_CORES):
        sl = slice(c * BS, (c + 1) * BS)
        xs = x[sl]  # (BS, CIN, H, W)
        xpad = np.zeros((BS, CIN, HP, WP), BF16)
        xpad[:, :, 1 : H + 1, 1 : W + 1] = xs.astype(BF16)
        zaT = np.concatenate(
            [z[sl].T, np.ones((1, BS), np.float32)], axis=0
        ).astype(BF16)  # (65, BS)
        in_maps.append(
            {
                "xa": xpad.reshape(BS * CIN, HP, WP),
                "zaT": zaT,
                "w1a": w1a,
                "w2a": w2a,
                "w3a": w3a,
            }
        )
    return in_maps


_NC_CACHE = {}
LAST_EXEC_NS = None
LAST_TRACE_DIR = None


def _get_nc():
    if "nc" not in _NC_CACHE:
        _NC_CACHE["nc"] = build_nc()
    return _NC_CACHE["nc"]


def kernel(x, z, w1, b1, w2, b2, w3, b3, _trace=False):
    global LAST_EXEC_NS, LAST_TRACE_DIR
    _install_ntff_hook()
    from concourse.bass_utils import run_bass_kernel_spmd

    nc = _get_nc()
    in_maps = _host_prep(x, z, w1, b1, w2, b2, w3, b3)
    kwargs = {}
    if _trace:
        import tempfile

        LAST_TRACE_DIR = tempfile.mkdtemp(prefix="adaptconv_trace_")
        kwargs = dict(trace=True, tmpdir=LAST_TRACE_DIR)
    res = run_bass_kernel_spmd(
        nc, in_maps, core_ids=list(range(N_CORES)), **kwargs
    )
    LAST_EXEC_NS = res.exec_time_ns
    RG = 16
    cores = []
    for i in range(N_CORES):
        arr = np.asarray(res.results[i]["out"])  # (H//RG, 2, 128, RG, W)
        y = (
            arr.reshape(H // RG, 2, 2, COUT, RG, W)
            .transpose(1, 2, 3, 0, 4, 5)
            .reshape(BS, COUT, H, W)
        )
        cores.append(y)
    return np.concatenate(cores, axis=0).astype(np.float32)
